# revision 15
# baseline (speedup 1.0000x reference)
"""Multi-head attention (nn_Attention_18528488915211) on 8 Trainium2 NeuronCores.

Sharding: tensor-parallel over heads. 16 heads / 8 cores = 2 heads per core.
Each core computes Q/K/V projections for its 256 columns of Wq/Wk/Wv,
attention for its 2 heads, and a partial output projection with its 256 rows
of Wo. The host sums the 8 partial outputs (the TP all-reduce) and adds bo.

Device kernel layout (fp32 storage, float32r matmuls, rel err ~2e-4):
  - x is fed transposed (xt [dmodel, tokens]) so projections need no
    on-device transpose (fp32 DMA-transpose is unsupported).
  - Q^T,K^T computed as [dhead, t] (weights stationary, Identity+bias
    drain on the otherwise-idle ACT engine); V natural [t, d].
  - Scores computed transposed: S^T[j,i] = K^T(lhsT) . Q^T(rhs); exp on
    ACT with the 1/128 scale folded in; AV keeps V stationary.
  - Softmax row-sums accumulate on the Vector engine (chained adds over
    the exp tiles), freeing the PE from rowsum matmuls; approximate
    reciprocal; per-query normalization of the attention output.
  - Output projection is fused into the attention loop per 512-token
    chunk so its PE work and DMA writes overlap attention's ACT/DVE time.
"""

import numpy as np

P = 128          # partitions
DM = 2048        # dmodel
DH = 128         # dhead
HPC = 2          # heads per core
DC = HPC * DH    # dmodel columns per core (256)
B = 4            # batch
L = 2048         # sequence length
T = B * L        # total tokens (8192)
KS = DM // P     # contraction subtiles (16)
TC = 512         # token chunk (matmul free dim)
NCORES = 8


def _build_nc():
    import concourse.mybir as mybir
    import concourse.tile as tile
    from concourse import bacc

    f32 = mybir.dt.float32
    f32r = mybir.dt.float32r
    EXP = mybir.ActivationFunctionType.Exp
    IDENT = mybir.ActivationFunctionType.Identity

    nc = bacc.Bacc("TRN2", target_bir_lowering=False, debug=False,
                   num_devices=NCORES)

    xt = nc.dram_tensor("xt", [DM, T], f32r, kind="ExternalInput").ap()
    wq = nc.dram_tensor("wq", [DM, DC], f32r, kind="ExternalInput").ap()
    wk = nc.dram_tensor("wk", [DM, DC], f32r, kind="ExternalInput").ap()
    wv = nc.dram_tensor("wv", [DM, DC], f32r, kind="ExternalInput").ap()
    bq = nc.dram_tensor("bq", [DC], f32, kind="ExternalInput").ap()
    bk = nc.dram_tensor("bk", [DC], f32, kind="ExternalInput").ap()
    bv = nc.dram_tensor("bv", [DC], f32, kind="ExternalInput").ap()
    wo = nc.dram_tensor("wo", [DC, DM], f32r, kind="ExternalInput").ap()
    out = nc.dram_tensor("out", [T, DM], f32, kind="ExternalOutput").ap()

    with tile.TileContext(nc) as tc:
        with (
            tc.tile_pool(name="wpool", bufs=1) as wpool,
            tc.tile_pool(name="xpool", bufs=18) as xpool,
            tc.tile_pool(name="qkv", bufs=1) as qkv,
            tc.tile_pool(name="ptp", bufs=3) as ptp,
            tc.tile_pool(name="misc", bufs=2) as misc,
            tc.tile_pool(name="ps", bufs=3, space="PSUM") as ps,
        ):
            # --- resident weights/constants (wo last: needed latest) ---
            wq_sb = wpool.tile([P, KS, DC], f32r, tag="wq")
            wk_sb = wpool.tile([P, KS, DC], f32r, tag="wk")
            wv_sb = wpool.tile([P, KS, DC], f32r, tag="wv")
            nc.sync.dma_start(wq_sb[:], wq.rearrange("(ks p) d -> p ks d", p=P))
            nc.sync.dma_start(wk_sb[:], wk.rearrange("(ks p) d -> p ks d", p=P))
            nc.sync.dma_start(wv_sb[:], wv.rearrange("(ks p) d -> p ks d", p=P))
            bq_sb = wpool.tile([P, HPC], f32, tag="bq")
            bk_sb = wpool.tile([P, HPC], f32, tag="bk")
            nc.sync.dma_start(bq_sb[:], bq.rearrange("(h d) -> d h", d=P))
            nc.sync.dma_start(bk_sb[:], bk.rearrange("(h d) -> d h", d=P))
            bv_sb = wpool.tile([P, DC], f32, tag="bv")
            nc.sync.dma_start(bv_sb[:], bv[None, :].to_broadcast((P, DC)))
            ones_f32 = wpool.tile([P, P], f32, tag="ones_f32")
            nc.any.memset(ones_f32[:], 1.0)
            ones_sb = wpool.tile([P, P], f32r, tag="ones")
            nc.vector.tensor_scalar_add(ones_sb[:], ones_f32[:], 0.0)
            wo_sb = wpool.tile([P, HPC, DM], f32r, tag="wo")
            nc.sync.dma_start(wo_sb[:], wo.rearrange("(h p) n -> p h n", p=P))

            for b in range(B):
                t0 = b * L
                qt_sb = qkv.tile([P, HPC, L], f32r, tag="qt", name="qt")
                kt_sb = qkv.tile([P, HPC, L], f32r, tag="kt", name="kt")
                v_sb = qkv.tile([P, L // P, DC], f32r, tag="v", name="v")
                ot_sb = qkv.tile([P, HPC, L], f32r, tag="ot", name="ot")

                # ============ Phase A: Q/K/V projections ============
                for tci in range(L // TC):
                    xts = []
                    for ks in range(KS):
                        xt_t = xpool.tile([P, TC], f32r, tag="xt")
                        nc.sync.dma_start(
                            xt_t[:],
                            xt[ks * P:(ks + 1) * P,
                               t0 + tci * TC: t0 + (tci + 1) * TC],
                        )
                        xts.append(xt_t)
                    for w_sb, o_sb, b_sb in ((wq_sb, qt_sb, bq_sb),
                                             (wk_sb, kt_sb, bk_sb)):
                        for h in range(HPC):
                            acc = ps.tile([P, TC], f32, tag="ps", name="qk")
                            for ks in range(KS):
                                nc.tensor.matmul(
                                    acc[:],
                                    w_sb[:, ks, h * DH:(h + 1) * DH],
                                    xts[ks][:],
                                    start=(ks == 0), stop=(ks == KS - 1),
                                )
                            # drain on ACT (idle during projections)
                            nc.scalar.activation(
                                o_sb[:, h, tci * TC:(tci + 1) * TC],
                                acc[:], IDENT, bias=b_sb[:, h:h + 1],
                            )
                    for tb in range(TC // P):
                        acc = ps.tile([P, TC], f32, tag="ps", name="vps")
                        for ks in range(KS):
                            nc.tensor.matmul(
                                acc[:, :DC],
                                xts[ks][:, tb * P:(tb + 1) * P],
                                wv_sb[:, ks, :],
                                start=(ks == 0), stop=(ks == KS - 1),
                            )
                        nc.vector.tensor_add(
                            v_sb[:, tci * (TC // P) + tb, :],
                            acc[:, :DC], bv_sb[:],
                        )

                # ===== Phase B+C: attention with fused output projection =====
                for ic in range(L // TC):
                    for h in range(HPC):
                        hd = slice(h * DH, (h + 1) * DH)
                        q_rhs = qt_sb[:, h, ic * TC:(ic + 1) * TC]
                        ot_ps = ps.tile([P, TC], f32, tag="ot",
                                        name="ot_ps", bufs=2)
                        racc = misc.tile([P, 2 * TC], f32r, tag="racc",
                                         name="racc", bufs=2)
                        racc_g = misc.tile([P, 2 * TC], f32, tag="raccg",
                                           name="racc_g", bufs=2)
                        pt_prev = None
                        for jp in range(L // P // 2):
                            pt2 = ptp.tile([P, 2 * TC], f32r, tag="pt",
                                           name="pt2")
                            for u in range(2):
                                js = 2 * jp + u
                                st_ps = ps.tile([P, TC], f32, tag="st",
                                                name="st_ps", bufs=3)
                                nc.tensor.matmul(
                                    st_ps[:],
                                    kt_sb[:, h, js * P:(js + 1) * P],
                                    q_rhs, start=True, stop=True,
                                )
                                nc.scalar.activation(
                                    pt2[:, u * TC:(u + 1) * TC], st_ps[:],
                                    EXP, scale=1.0 / DH,
                                )
                                nc.tensor.matmul(
                                    ot_ps[:], v_sb[:, js, hd],
                                    pt2[:, u * TC:(u + 1) * TC],
                                    start=(js == 0), stop=(js == L // P - 1),
                                )
                            # rowsum partials: jp 0-4 on DVE, 5-7 on the
                            # otherwise-idle GpSimd engine
                            if jp in (0, 5):
                                pt_prev = pt2
                            elif jp == 1:
                                nc.vector.tensor_add(racc[:], pt_prev[:],
                                                     pt2[:])
                            elif jp in (2, 3, 4):
                                nc.vector.tensor_add(racc[:], racc[:],
                                                     pt2[:])
                            elif jp == 6:
                                nc.gpsimd.tensor_add(racc_g[:], pt_prev[:],
                                                     pt2[:])
                            else:
                                nc.gpsimd.tensor_add(racc_g[:], racc_g[:],
                                                     pt2[:])
                        nc.vector.tensor_add(racc[:], racc[:],
                                              racc_g[:])
                        rs_ps = ps.tile([P, TC], f32, tag="st",
                                        name="rs_ps", bufs=3)
                        nc.tensor.matmul(rs_ps[:], ones_sb[:], racc[:, :TC],
                                         start=True, stop=False)
                        nc.tensor.matmul(rs_ps[:], ones_sb[:], racc[:, TC:],
                                         start=False, stop=True)
                        rcp = misc.tile([P, TC], f32, tag="rcp", name="rcp",
                                        bufs=2)
                        nc.vector.reciprocal_approx_fast(rcp[:], rs_ps[:])
                        nc.vector.tensor_mul(
                            ot_sb[:, h, ic * TC:(ic + 1) * TC],
                            ot_ps[:], rcp[:],
                        )
                    # fused partial output projection for this token chunk
                    for tbl in range(TC // P):
                        tb = ic * (TC // P) + tbl
                        for ncl in range(DM // TC):
                            o_ps = ps.tile([P, TC], f32, tag="ps",
                                           name="o_ps")
                            for h in range(HPC):
                                nc.tensor.matmul(
                                    o_ps[:],
                                    ot_sb[:, h, tb * P:(tb + 1) * P],
                                    wo_sb[:, h, ncl * TC:(ncl + 1) * TC],
                                    start=(h == 0), stop=(h == HPC - 1),
                                )
                            o_out = misc.tile([P, TC], f32, tag="oout",
                                              name="oout", bufs=3)
                            nc.any.tensor_copy(o_out[:], o_ps[:])
                            nc.sync.dma_start(
                                out[t0 + tb * P: t0 + (tb + 1) * P,
                                    ncl * TC:(ncl + 1) * TC],
                                o_out[:],
                            )

    nc.compile()
    return nc


_NC_CACHE = None


def kernel(**inputs: np.ndarray) -> np.ndarray:
    from concourse.bass_utils import run_bass_kernel_spmd

    global _NC_CACHE
    x = np.asarray(inputs["x"], dtype=np.float32)
    Wq, bq = np.asarray(inputs["Wq"]), np.asarray(inputs["bq"])
    Wk, bk = np.asarray(inputs["Wk"]), np.asarray(inputs["bk"])
    Wv, bv = np.asarray(inputs["Wv"]), np.asarray(inputs["bv"])
    Wo, bo = np.asarray(inputs["Wo"]), np.asarray(inputs["bo"])

    xt = np.ascontiguousarray(x.reshape(T, DM).T)

    in_maps = []
    for c in range(NCORES):
        sl = slice(c * DC, (c + 1) * DC)
        in_maps.append({
            "xt": xt,
            "wq": np.ascontiguousarray(Wq[:, sl]),
            "wk": np.ascontiguousarray(Wk[:, sl]),
            "wv": np.ascontiguousarray(Wv[:, sl]),
            "bq": np.ascontiguousarray(bq[sl]),
            "bk": np.ascontiguousarray(bk[sl]),
            "bv": np.ascontiguousarray(bv[sl]),
            "wo": np.ascontiguousarray(Wo[sl, :]),
        })

    if _NC_CACHE is None:
        _NC_CACHE = _build_nc()
    res = run_bass_kernel_spmd(_NC_CACHE, in_maps, core_ids=list(range(NCORES)))

    acc = res.results[0]["out"].astype(np.float32)
    for c in range(1, NCORES):
        acc = acc + res.results[c]["out"]
    acc = acc + bo[None, :].astype(np.float32)
    return acc.reshape(B, L, DM)
